# revision 8
# baseline (speedup 1.0000x reference)
"""CLUB loss kernel for Trainium2, 8 NeuronCores (SPMD data-parallel), v2.

Math: with flat_x (N,d), iv = exp(-p_logvar):
  positive_i = -0.5 * sum_d (x_i - mu_i)^2 * iv_i
  negative_i = -0.5 * sum_d iv_i * (ex2 - 2 mu_i ex + mu_i^2),  ex/ex2 = col-moments of flat_x
  loss = mean_i(positive_i - negative_i)
Decomposed into global sums (single pass over data):
  sx[d]  = sum_i x,  sxx[d] = sum_i x^2
  A[d]   = sum_i iv, B2[d]  = sum_i iv*mu
  T      = sum_{i,d} (iv*x^2 - 2*iv*mu*x)
  loss = -0.5/N * [ T - dot(sxx,A)/N + dot(sx,B2*2)/N ]

v2 design (i-major, fp16-heavy, PE-reduced):
- Host pre-flattens x to flat_x rows (N,128) fp32 and casts mu/logvar to
  fp16 (x stays fp32: casting x to fp16 is the dominant precision loss --
  x appears squared, and f16(x16*x16) rounding is *biased* through the
  near-cancelling loss structure; measured 1.8e-3 rel err for this split
  vs 9.9e-3+ for all-fp16 variants; gate is 2e-2).
- All tensors land i-major (partition = row) pair-packed so every DMA run
  is >= 512B (256-row blocks: partition p holds rows blk*256+2p, +1).
- Products on ACT/DVE/Pool; ALL six column reductions run on the PE as
  ones-vector matmuls accumulating into fp32 PSUM (DVE tensor_reduce has
  no 2x fp16 mode and was the 55us bottleneck of v1; a ones-matmul
  consumes 128 rows/cycle at fp16).
- Per-core output is a (6,512) stats block of PSUM partials; the host
  folds partials, sums the 8 cores, and finishes the O(d) combine in
  fp64 (same contract as v1).
Engine budget per core (8192 rows): DMA 23us | DVE ~21 | ACT ~21 |
PE ~21 | Pool ~16.
"""

import numpy as np

B, D, H, W = 16, 128, 64, 64
N = B * H * W            # 65536
NCORES = 8
ROWS = N // NCORES       # 8192 rows per core
CHUNK = 1024             # rows per pipeline chunk
NCHUNK = ROWS // CHUNK   # 4 chunks
SLICE = 512              # matmul rhs free-size per accumulation step
# free layout per chunk: (blk, two, d) with blk = CHUNK//256, two = 2
CBLK = CHUNK // 256      # 8 pair-blocks per chunk

_CACHE = {}

# stats row order in the (5, 512) output block (Ta comes from tad)
ST_A, ST_SXX, ST_B2, ST_TB, ST_SX = range(5)


def _build_nc():
    import concourse.bacc as bacc
    import concourse.mybir as mybir
    from concourse.tile import TileContext

    f32 = mybir.dt.float32
    f16 = mybir.dt.float16
    ALU = mybir.AluOpType
    AF = mybir.ActivationFunctionType

    nc = bacc.Bacc(num_devices=NCORES)
    x_in = nc.dram_tensor("x", [ROWS, D], f32, kind="ExternalInput")
    mu_in = nc.dram_tensor("p_mu", [ROWS, D], f16, kind="ExternalInput")
    lv_in = nc.dram_tensor("p_logvar", [ROWS, D], f16, kind="ExternalInput")
    stats_out = nc.dram_tensor("stats", [1, 5 * SLICE], f32,
                               kind="ExternalOutput")
    tad_out = nc.dram_tensor("tad", [128, 128], f32, kind="ExternalOutput")

    with TileContext(nc) as tc:
        with (
            tc.tile_pool(name="const", bufs=1) as constp,
            tc.tile_pool(name="xin", bufs=3) as xinp,
            tc.tile_pool(name="muin", bufs=3) as muinp,
            tc.tile_pool(name="work", bufs=3) as work,
            tc.tile_pool(name="ps", bufs=1, space="PSUM") as psp,
        ):
            ones = constp.tile([128, 1], f16, name="ones")
            nc.vector.memset(ones[:], 1.0)

            # five persistent PSUM accumulators + the Ta diag matrix
            accs = [psp.tile([1, SLICE], f32, name=f"acc{i}") for i in range(5)]
            taD = psp.tile([128, 128], f32, name="taD")

            for c in range(NCHUNK):
                r0 = c * CHUNK
                x_c = xinp.tile([128, CHUNK], f32, tag="x_c", name="x_c")
                mu_c = muinp.tile([128, CHUNK], f16, tag="mu_c", name="mu_c")
                lv_c = muinp.tile([128, CHUNK], f16, tag="lv_c", name="lv_c")
                # pair-packed i-major load: partition p <- rows blk*256+2p(+1)
                # so each partition receives contiguous (two, d) runs
                for t, src in ((x_c, x_in), (mu_c, mu_in), (lv_c, lv_in)):
                    nc.sync.dma_start(
                        out=t[:].rearrange("p (blk two d) -> p blk two d",
                                           two=2, d=D),
                        in_=src[r0:r0 + CHUNK, :].rearrange(
                            "(blk p two) d -> p blk two d", p=128, two=2),
                    )

                iv_c = work.tile([128, CHUNK], f16, tag="iv", name="iv")
                xsq_c = work.tile([128, CHUNK], f16, tag="xsq", name="xsq")
                x16_c = work.tile([128, CHUNK], f16, tag="x16", name="x16")
                j_c = work.tile([128, CHUNK], f16, tag="j", name="j")
                t2_c = work.tile([128, CHUNK], f16, tag="t2", name="t2")

                def stream(acc, src):
                    # ones^T @ src accumulates column sums into PSUM; acc
                    # col layout after full accumulation: (blk%2, two, d)
                    for s in range(CHUNK // SLICE):
                        nc.tensor.matmul(
                            acc[:], ones[:],
                            src[:, s * SLICE:(s + 1) * SLICE],
                            start=(c == 0 and s == 0),
                            stop=(c == NCHUNK - 1 and s == CHUNK // SLICE - 1),
                            skip_group_check=True)

                # ACT: iv = exp(-lv); xsq = x^2 (fp32 source: unbiased
                # f16 rounding); Pool: x16 = f16 copy of x
                nc.scalar.activation(iv_c[:], lv_c[:], AF.Exp,
                                     bias=0.0, scale=-1.0)
                nc.scalar.square(xsq_c[:], x_c[:])
                nc.gpsimd.tensor_copy(x16_c[:], x_c[:])
                # early PE streams: ready as soon as ACT finishes, so the
                # in-order PE queue never stalls behind late products
                stream(accs[ST_A], iv_c)
                stream(accs[ST_SXX], xsq_c)
                # Ta = sum(iv*x^2) via diag accumulation: PE multiplies
                # xsq*iv exactly into fp32 PSUM (no f16 product rounding);
                # host reads the diagonal of taD
                for b in range(CHUNK // 128):
                    bl = slice(b * 128, (b + 1) * 128)
                    nc.tensor.matmul(taD[:], xsq_c[:, bl], iv_c[:, bl],
                                     start=(c == 0 and b == 0),
                                     stop=(c == NCHUNK - 1
                                           and b == CHUNK // 128 - 1),
                                     skip_group_check=True)
                stream(accs[ST_SX], x16_c)
                # DVE: j = iv*mu, then t2 = j*x16 (all-f16, 2x mode)
                nc.vector.tensor_tensor(j_c[:], iv_c[:], mu_c[:], ALU.mult)
                nc.vector.tensor_tensor(t2_c[:], j_c[:], x16_c[:], ALU.mult)
                # late PE streams
                stream(accs[ST_B2], j_c)
                stream(accs[ST_TB], t2_c)

            # PSUM cannot be DMA'd directly: stage through SBUF
            # (Pool has no PSUM port; alternate ACT/DVE for the copies)
            st = constp.tile([1, 5 * SLICE], f32, name="st")
            for i, a in enumerate(accs):
                eng = (nc.scalar.copy, nc.vector.tensor_copy)[i % 2]
                eng(st[:, i * SLICE:(i + 1) * SLICE], a[:])
            tas = constp.tile([128, 128], f32, name="tas")
            nc.scalar.copy(tas[:], taD[:])
            nc.sync.dma_start(out=stats_out[:], in_=st[:])
            nc.sync.dma_start(out=tad_out[:], in_=tas[:])

    return nc


def get_nc(use_collective=True, stats_output=True):
    key = "nc_v2"
    if key not in _CACHE:
        nc = _build_nc()
        if not nc.is_finalized():
            nc.finalize()
        _CACHE[key] = nc
    return _CACHE[key]


def make_in_maps(x, p_mu, p_logvar):
    x = np.asarray(x)
    flat_x = np.ascontiguousarray(
        np.transpose(x, (0, 2, 3, 1)).reshape(N, D).astype(np.float32))
    mu16 = np.ascontiguousarray(np.asarray(p_mu).astype(np.float16))
    lv16 = np.ascontiguousarray(np.asarray(p_logvar).astype(np.float16))
    in_maps = []
    for c in range(NCORES):
        r = slice(c * ROWS, (c + 1) * ROWS)
        in_maps.append({
            "x": flat_x[r],
            "p_mu": mu16[r],
            "p_logvar": lv16[r],
        })
    return in_maps


MODE = "host"


def kernel(x, p_mu, p_logvar):
    from concourse.bass_utils import run_bass_kernel_spmd

    in_maps = make_in_maps(x, p_mu, p_logvar)
    nc = get_nc()
    res = run_bass_kernel_spmd(nc, in_maps, list(range(NCORES)))
    s = np.zeros((1, 5 * SLICE), dtype=np.float64)
    Ta = 0.0
    for c in range(NCORES):
        s += np.asarray(res.results[c]["stats"], dtype=np.float64)
        Ta += np.trace(np.asarray(res.results[c]["tad"], dtype=np.float64))
    # each stats row holds (blk%2, two, d) partials of the column sums
    part = s.reshape(5, SLICE // D, D).sum(axis=1)   # -> (5, 128)
    A, sxx, B2p, Tb_v, sx = (part[i] for i in range(5))
    T = Ta - 2.0 * Tb_v.sum()
    loss = -0.5 / N * (T - sxx.dot(A) / N + sx.dot(2.0 * B2p) / N)
    return np.asarray(loss, dtype=np.float32).reshape(())


# revision 9
# speedup vs baseline: 1.1537x; 1.1537x over previous
"""CLUB loss kernel for Trainium2, 8 NeuronCores (SPMD data-parallel), v2.

Math: with flat_x (N,d), iv = exp(-p_logvar):
  positive_i = -0.5 * sum_d (x_i - mu_i)^2 * iv_i
  negative_i = -0.5 * sum_d iv_i * (ex2 - 2 mu_i ex + mu_i^2),  ex/ex2 = col-moments of flat_x
  loss = mean_i(positive_i - negative_i)
Decomposed into global sums (single pass over data):
  sx[d]  = sum_i x,  sxx[d] = sum_i x^2
  A[d]   = sum_i iv, B2[d]  = sum_i iv*mu
  T      = sum_{i,d} (iv*x^2 - 2*iv*mu*x)
  loss = -0.5/N * [ T - dot(sxx,A)/N + dot(sx,B2*2)/N ]

v2 design (i-major, fp16-heavy, PE-reduced):
- Host pre-flattens x to flat_x rows (N,128) fp32 and casts mu/logvar to
  fp16 (x stays fp32: casting x to fp16 is the dominant precision loss --
  x appears squared, and f16(x16*x16) rounding is *biased* through the
  near-cancelling loss structure; measured 1.8e-3 rel err for this split
  vs 9.9e-3+ for all-fp16 variants; gate is 2e-2).
- All tensors land i-major (partition = row) pair-packed so every DMA run
  is >= 512B (256-row blocks: partition p holds rows blk*256+2p, +1).
- Products on ACT/DVE/Pool; ALL six column reductions run on the PE as
  ones-vector matmuls accumulating into fp32 PSUM (DVE tensor_reduce has
  no 2x fp16 mode and was the 55us bottleneck of v1; a ones-matmul
  consumes 128 rows/cycle at fp16).
- Per-core output is a (6,512) stats block of PSUM partials; the host
  folds partials, sums the 8 cores, and finishes the O(d) combine in
  fp64 (same contract as v1).
Engine budget per core (8192 rows): DMA 23us | DVE ~21 | ACT ~21 |
PE ~21 | Pool ~16.
"""

import numpy as np

B, D, H, W = 16, 128, 64, 64
N = B * H * W            # 65536
NCORES = 8
ROWS = N // NCORES       # 8192 rows per core
CHUNK = 1024             # rows per pipeline chunk
NCHUNK = ROWS // CHUNK   # 4 chunks
SLICE = 512              # matmul rhs free-size per accumulation step
# free layout per chunk: (blk, two, d) with blk = CHUNK//256, two = 2
CBLK = CHUNK // 256      # 8 pair-blocks per chunk

_CACHE = {}

# stats row order in the (5, 512) output block (Ta comes from tad)
ST_A, ST_SXX, ST_B2, ST_TB, ST_SX = range(5)


def _build_nc():
    import concourse.bacc as bacc
    import concourse.mybir as mybir
    from concourse.tile import TileContext

    f32 = mybir.dt.float32
    f16 = mybir.dt.float16
    ALU = mybir.AluOpType
    AF = mybir.ActivationFunctionType

    nc = bacc.Bacc(num_devices=NCORES)
    x_in = nc.dram_tensor("x", [ROWS, D], f32, kind="ExternalInput")
    mu_in = nc.dram_tensor("p_mu", [ROWS, D], f16, kind="ExternalInput")
    lv_in = nc.dram_tensor("p_logvar", [ROWS, D], f16, kind="ExternalInput")
    stats_out = nc.dram_tensor("stats", [1, 5 * SLICE], f32,
                               kind="ExternalOutput")
    tad_out = nc.dram_tensor("tad", [128, 128], f32, kind="ExternalOutput")

    with TileContext(nc) as tc:
        with (
            tc.tile_pool(name="const", bufs=1) as constp,
            tc.tile_pool(name="xin", bufs=3) as xinp,
            tc.tile_pool(name="muin", bufs=3) as muinp,
            tc.tile_pool(name="work", bufs=3) as work,
            tc.tile_pool(name="ps", bufs=1, space="PSUM") as psp,
        ):
            ones = constp.tile([128, 1], f16, name="ones")
            nc.vector.memset(ones[:], 1.0)

            # five persistent PSUM accumulators + the Ta diag matrix
            accs = [psp.tile([1, SLICE], f32, name=f"acc{i}") for i in range(5)]
            taD = psp.tile([128, 128], f32, name="taD")

            for c in range(NCHUNK):
                r0 = c * CHUNK
                x_c = xinp.tile([128, CHUNK], f32, tag="x_c", name="x_c")
                mu_c = muinp.tile([128, CHUNK], f16, tag="mu_c", name="mu_c")
                lv_c = muinp.tile([128, CHUNK], f16, tag="lv_c", name="lv_c")
                # pair-packed i-major load: partition p <- rows blk*256+2p(+1)
                # so each partition receives contiguous (two, d) runs
                for t, src in ((x_c, x_in), (mu_c, mu_in), (lv_c, lv_in)):
                    nc.sync.dma_start(
                        out=t[:].rearrange("p (blk two d) -> p blk two d",
                                           two=2, d=D),
                        in_=src[r0:r0 + CHUNK, :].rearrange(
                            "(blk p two) d -> p blk two d", p=128, two=2),
                    )

                iv_c = work.tile([128, CHUNK], f16, tag="iv", name="iv")
                xsq_c = work.tile([128, CHUNK], f16, tag="xsq", name="xsq")
                x16_c = work.tile([128, CHUNK], f16, tag="x16", name="x16")
                j_c = work.tile([128, CHUNK], f16, tag="j", name="j")
                t2_c = work.tile([128, CHUNK], f16, tag="t2", name="t2")

                def stream(acc, src):
                    # ones^T @ src accumulates column sums into PSUM; acc
                    # col layout after full accumulation: (blk%2, two, d)
                    for s in range(CHUNK // SLICE):
                        nc.tensor.matmul(
                            acc[:], ones[:],
                            src[:, s * SLICE:(s + 1) * SLICE],
                            start=(c == 0 and s == 0),
                            stop=(c == NCHUNK - 1 and s == CHUNK // SLICE - 1),
                            skip_group_check=True)

                # ACT: iv = exp(-lv); xsq = x^2 (fp32 source: unbiased
                # f16 rounding); Pool: x16 = f16 copy of x
                nc.scalar.activation(iv_c[:], lv_c[:], AF.Exp,
                                     bias=0.0, scale=-1.0)
                nc.scalar.square(xsq_c[:], x_c[:])
                nc.scalar.copy(x16_c[:], x_c[:])
                # early PE streams: ready as soon as ACT finishes, so the
                # in-order PE queue never stalls behind late products
                stream(accs[ST_A], iv_c)
                stream(accs[ST_SXX], xsq_c)
                stream(accs[ST_SX], x16_c)
                # DVE: j = iv*mu (all-f16, 2x mode); Pool: t2 = j*x16
                nc.vector.tensor_tensor(j_c[:], iv_c[:], mu_c[:], ALU.mult)
                nc.gpsimd.tensor_tensor(t2_c[:], j_c[:], x16_c[:], ALU.mult)
                stream(accs[ST_B2], j_c)
                stream(accs[ST_TB], t2_c)
                # Ta = sum(iv*x^2) via diag accumulation: PE multiplies
                # xsq*iv exactly into fp32 PSUM (no f16 product rounding);
                # host reads the diagonal of taD. Placed last per chunk so
                # the in-order PE queue is never blocked by them
                for b in range(CHUNK // 128):
                    bl = slice(b * 128, (b + 1) * 128)
                    nc.tensor.matmul(taD[:], xsq_c[:, bl], iv_c[:, bl],
                                     start=(c == 0 and b == 0),
                                     stop=(c == NCHUNK - 1
                                           and b == CHUNK // 128 - 1),
                                     skip_group_check=True)

            # PSUM cannot be DMA'd directly: stage through SBUF
            # (Pool has no PSUM port; alternate ACT/DVE for the copies)
            st = constp.tile([1, 5 * SLICE], f32, name="st")
            for i, a in enumerate(accs):
                eng = (nc.scalar.copy, nc.vector.tensor_copy)[i % 2]
                eng(st[:, i * SLICE:(i + 1) * SLICE], a[:])
            tas = constp.tile([128, 128], f32, name="tas")
            nc.scalar.copy(tas[:], taD[:])
            nc.sync.dma_start(out=stats_out[:], in_=st[:])
            nc.sync.dma_start(out=tad_out[:], in_=tas[:])

    return nc


def get_nc(use_collective=True, stats_output=True):
    key = "nc_v2"
    if key not in _CACHE:
        nc = _build_nc()
        if not nc.is_finalized():
            nc.finalize()
        _CACHE[key] = nc
    return _CACHE[key]


def make_in_maps(x, p_mu, p_logvar):
    x = np.asarray(x)
    flat_x = np.ascontiguousarray(
        np.transpose(x, (0, 2, 3, 1)).reshape(N, D).astype(np.float32))
    mu16 = np.ascontiguousarray(np.asarray(p_mu).astype(np.float16))
    lv16 = np.ascontiguousarray(np.asarray(p_logvar).astype(np.float16))
    in_maps = []
    for c in range(NCORES):
        r = slice(c * ROWS, (c + 1) * ROWS)
        in_maps.append({
            "x": flat_x[r],
            "p_mu": mu16[r],
            "p_logvar": lv16[r],
        })
    return in_maps


MODE = "host"


def kernel(x, p_mu, p_logvar):
    from concourse.bass_utils import run_bass_kernel_spmd

    in_maps = make_in_maps(x, p_mu, p_logvar)
    nc = get_nc()
    res = run_bass_kernel_spmd(nc, in_maps, list(range(NCORES)))
    s = np.zeros((1, 5 * SLICE), dtype=np.float64)
    Ta = 0.0
    for c in range(NCORES):
        s += np.asarray(res.results[c]["stats"], dtype=np.float64)
        Ta += np.trace(np.asarray(res.results[c]["tad"], dtype=np.float64))
    # each stats row holds (blk%2, two, d) partials of the column sums
    part = s.reshape(5, SLICE // D, D).sum(axis=1)   # -> (5, 128)
    A, sxx, B2p, Tb_v, sx = (part[i] for i in range(5))
    T = Ta - 2.0 * Tb_v.sum()
    loss = -0.5 / N * (T - sxx.dot(A) / N + sx.dot(2.0 * B2p) / N)
    return np.asarray(loss, dtype=np.float32).reshape(())
